# revision 20
# baseline (speedup 1.0000x reference)
"""Trainium2 Bass kernel for the attention-decoder recurrence.

Problem: B=128 batch, T=128 steps, U=M=64. A 127-step sequential scan of
(Bahdanau-attention -> 1-dim projection -> LSTM cell), then a final
attention readout. Returns (h_f (128,64), ctx (128,1,64)).

Sharding: data-parallel over batch, 16 per NeuronCore x 8 cores, all
parameters replicated, no cross-device traffic inside the recurrence.

Per-core structure (see kernel-side comments):
 - The 16-sample slice is further split into two independent groups of 8
   whose serial dependency chains interleave on the engines (software
   pipelining across the step's ~45 small ops).
 - Recurrent state per group: hs (128,8) SBUF, rows 0:64 H=2h
   (transposed), rows 64:128 S=2c. The factor-2 state convention turns
   every sigmoid into 0.5*tanh(z/2)+0.5 with the 0.5s folded into
   host-prescaled weights, so ScalarE only ever needs the exp/tanh table
   set (no ~2.7us table switches).
 - r2 = h_en@Ue is host-precomputed and staged into PSUM in a
   (p=(b%2)*64+u, k*128+t) layout; each per-step tanh ACT op reads one
   128x128 chunk with a per-partition bias column q (fused add+tanh).
 - e = ve . tanh(...) via PE matmuls with a zero-padded bf16 stationary
   operand; softmax needs no max-subtraction (|e| <= sum|ve| ~ 5).
 - x = y*Wp0 + bp + (sum exp*hw)/(sum exp) uses fused exp+accum (s1),
   fused scalar_tensor_tensor+accum (s2), and a two-scalar tensor_scalar.
 - LSTM gate preactivations via one K=65 stacked matmul per gate pair.
"""
import numpy as np

B, T, U, M = 128, 128, 64, 64
N_CORES = 8
BS = B // N_CORES   # 16 batch per core
G = 2               # pipeline groups per core
GB = BS // G        # 8 batch per group
KCH = BS // 2       # 8 tanh chunks total per core
GCH = KCH // G      # 4 chunks per group

_prog_cache = {}


def _imports():
    import concourse.bass as bass
    import concourse.tile as tile
    from concourse import bacc, mybir
    from concourse.bass_utils import run_bass_kernel_spmd
    return bass, tile, bacc, mybir, run_bass_kernel_spmd


# ---------------------------------------------------------------- host prep
def host_prep(inputs, core):
    """Host-side numpy preprocessing for one core's input map."""
    import ml_dtypes
    sl = slice(core * BS, (core + 1) * BS)
    h_en = np.asarray(inputs["h_en_all"][sl], np.float32)   # (16, 128, 64)
    y = np.asarray(inputs["y"][sl, :, 0], np.float32)       # (16, 128)
    s0 = np.asarray(inputs["s"][sl], np.float32)            # (16, 64)
    h0 = np.asarray(inputs["h"][sl], np.float32)            # (16, 64)
    We = np.asarray(inputs["We"], np.float32)               # (128, 64)
    Ue = np.asarray(inputs["Ue"], np.float32)               # (64, 64)
    ve = np.asarray(inputs["ve"], np.float32)[:, 0]         # (64,)
    Wp = np.asarray(inputs["Wp"], np.float32)[:, 0]         # (65,)
    bp = float(np.asarray(inputs["bp"], np.float32)[0])
    Wk = np.asarray(inputs["Wk"], np.float32)               # (1, 256)
    Wr = np.asarray(inputs["Wr"], np.float32)               # (64, 256)
    b = np.asarray(inputs["b"], np.float32)                 # (256,)
    assert np.abs(b).max() == 0.0, "nonzero LSTM bias not supported"

    d = {}
    r2 = np.einsum("btm,mu->btu", h_en, Ue).astype(np.float32)
    r2_l = np.empty((128, 1024), np.float32)
    for k in range(KCH):
        for bb in range(2):
            r2_l[bb * 64:(bb + 1) * 64, k * 128:(k + 1) * 128] = r2[2 * k + bb].T
    d["r2_l"] = r2_l
    d["hen_sb"] = np.ascontiguousarray(h_en.transpose(1, 0, 2).reshape(T, BS * M))
    d["we2"] = np.ascontiguousarray(We * 0.5)
    # zero-padded ve blocks, per group: (128, GCH*GB) and the (128,2) final
    ve_blkz = np.zeros((128, KCH * GB), np.float32)
    ve_blk2 = np.zeros((128, 2), np.float32)
    for p in range(128):
        bb, u = p // 64, p % 64
        for k in range(KCH):          # global chunk k covers b = 2k, 2k+1
            g, kl = divmod(k, GCH)
            j = 2 * k + bb - GB * g   # local batch index within group
            ve_blkz[p, (g * GCH + kl) * GB + j] = ve[u]
        ve_blk2[p, bb] = ve[u]
    d["ve_blkz"] = ve_blkz.astype(ml_dtypes.bfloat16)
    d["ve_blk2"] = ve_blk2.astype(ml_dtypes.bfloat16)
    wra = Wr[:, 0:128] * 0.25
    wrb = np.concatenate([Wr[:, 128:192] * 0.5, Wr[:, 192:256] * 0.25], axis=1)
    wka = Wk[:, 0:128] * 0.5
    wkb = np.concatenate([Wk[:, 128:192], Wk[:, 192:256] * 0.5], axis=1)
    d["wrak"] = np.concatenate([wra, wka], axis=0).astype(np.float32)  # (65,128)
    d["wrbk"] = np.concatenate([wrb, wkb], axis=0).astype(np.float32)  # (65,128)
    d["hw_t"] = np.einsum("btm,m->bt", h_en, Wp[1:]).astype(np.float32)
    d["yp"] = (y * Wp[0] + bp).astype(np.float32)
    d["hs0"] = np.concatenate([2.0 * h0.T, 2.0 * s0.T], axis=0).astype(np.float32)
    return d


# ---------------------------------------------------------------- program
def build_program(n_steps=T - 1):
    bass, tile, bacc, mybir, _ = _imports()
    from contextlib import ExitStack
    F32 = mybir.dt.float32
    BF16 = mybir.dt.bfloat16
    AF = mybir.ActivationFunctionType
    OP = mybir.AluOpType

    nc = bacc.Bacc("TRN2", target_bir_lowering=False, debug=False,
                   num_devices=N_CORES)

    def din(name, shape, dt=F32):
        return nc.dram_tensor(name, list(shape), dt, kind="ExternalInput").ap()

    r2_d = din("r2_l", (128, 1024))
    hen_d = din("hen_sb", (128, 1024))
    we2_d = din("we2", (128, 64))
    vbz_d = din("ve_blkz", (128, KCH * GB), BF16)
    vb2_d = din("ve_blk2", (128, 2), BF16)
    wrak_d = din("wrak", (65, 128))
    wrbk_d = din("wrbk", (65, 128))
    hw_d = din("hw_t", (16, 128))
    yp_d = din("yp", (16, 128))
    hs0_d = din("hs0", (128, 16))
    hout_d = nc.dram_tensor("h_out", [BS, U], F32, kind="ExternalOutput").ap()
    ctxout_d = nc.dram_tensor("ctx_out", [BS, M], F32, kind="ExternalOutput").ap()

    with tile.TileContext(nc) as tc, ExitStack() as ctx:
        stat = ctx.enter_context(tc.tile_pool(name="stat", bufs=1))
        sc = ctx.enter_context(tc.tile_pool(name="sc", bufs=2))
        ps_q = ctx.enter_context(tc.tile_pool(name="ps_q", bufs=1, space="PSUM"))
        ps_e = ctx.enter_context(tc.tile_pool(name="ps_e", bufs=2, space="PSUM"))
        ps_z = ctx.enter_context(tc.tile_pool(name="ps_z", bufs=1, space="PSUM"))

        # ---- static loads
        r2 = stat.tile([128, 1024], F32)
        nc.sync.dma_start(r2[:], r2_d[:])
        hen = stat.tile([128, 1024], F32)
        nc.sync.dma_start(hen[:], hen_d[:])
        we2 = stat.tile([128, 64], F32)
        nc.sync.dma_start(we2[:], we2_d[:])
        vbz = stat.tile([128, KCH * GB], BF16)
        nc.sync.dma_start(vbz[:], vbz_d[:])
        vb2 = stat.tile([128, 2], BF16)
        nc.sync.dma_start(vb2[:], vb2_d[:])
        wrak = stat.tile([65, 128], F32)
        nc.sync.dma_start(wrak[:], wrak_d[:])
        wrbk = stat.tile([65, 128], F32)
        nc.sync.dma_start(wrbk[:], wrbk_d[:])

        # HAM warm-up: a dense burst of matmuls so the PE clock-gate opens
        # (K=8/8); steady-state MM duty afterwards is enough to retain it.
        for w in range(14):
            e_warm = ps_e.tile([GB, 128], F32, tag=f"e{w % G}", name=f"ew{w}")
            nc.tensor.matmul(e_warm[:],
                             r2[:, (w % 8) * 128:(w % 8) * 128 + GB],
                             r2[:, (w % 8) * 128:((w % 8) + 1) * 128],
                             start=True, stop=True)

        grp = []
        for g in range(G):
            gd = {}
            gd["hw"] = stat.tile([GB, 128], F32, tag=f"hw{g}", name=f"hw{g}")
            nc.sync.dma_start(gd["hw"][:], hw_d[g * GB:(g + 1) * GB, :])
            gd["yp"] = stat.tile([GB, 128], F32, tag=f"yp{g}", name=f"yp{g}")
            nc.sync.dma_start(gd["yp"][:], yp_d[g * GB:(g + 1) * GB, :])
            gd["hs"] = stat.tile([128, GB], F32, tag=f"hs{g}", name=f"hs{g}")
            nc.sync.dma_start(gd["hs"][:], hs0_d[:, g * GB:(g + 1) * GB])
            gd["xblk"] = stat.tile([32, 32], F32, tag=f"xblk{g}", name=f"xblk{g}")
            nc.gpsimd.memset(gd["xblk"][:], 0.0)
            gd["hsx"] = stat.tile([96, 32], F32, tag=f"hsx{g}", name=f"hsx{g}")
            grp.append(gd)

        def q_and_tanh(g):
            """q even/odd MMs, psum->sbuf copy, GCH tanh chunks -> th (bf16)."""
            gd = grp[g]
            hs = gd["hs"]
            q2 = ps_q.tile([128, GB // 2], F32, tag=f"q{g}")
            nc.tensor.matmul(q2[0:64, :], we2[:], hs[:, 0::2], start=True, stop=True)
            nc.tensor.matmul(q2[64:128, :], we2[:], hs[:, 1::2], start=True, stop=True)
            q_sb = sc.tile([128, GB // 2], F32, tag=f"qs{g}")
            nc.vector.tensor_copy(q_sb[:], q2[:])
            th = sc.tile([128, GCH * 128], BF16, tag=f"th{g}")
            for kl in range(GCH):
                kg = g * GCH + kl
                nc.scalar.activation(th[:, kl * 128:(kl + 1) * 128],
                                     r2[:, kg * 128:(kg + 1) * 128],
                                     AF.Tanh, bias=q_sb[:, kl:kl + 1], scale=1.0)
            return th

        def ve_contract(g, th):
            e_ps = ps_e.tile([GB, 128], F32, tag=f"e{g}")
            for kl in range(GCH):
                nc.tensor.matmul(
                    e_ps[:], vbz[:, (g * GCH + kl) * GB:(g * GCH + kl + 1) * GB],
                    th[:, kl * 128:(kl + 1) * 128],
                    start=(kl == 0), stop=(kl == GCH - 1))
            return e_ps

        # =================== the scan ===================
        for t in range(n_steps):
            for g in range(G):
                gd = grp[g]
                hs, xblk, hsx = gd["hs"], gd["xblk"], gd["hsx"]
                nc.vector.tensor_copy(hsx[0:64, 0:GB], hs[0:64, :])
                th = q_and_tanh(g)
                e_ps = ve_contract(g, th)

                exp_e = sc.tile([GB, 128], F32, tag=f"ex{g}")
                s1 = sc.tile([GB, 1], F32, tag=f"s1{g}")
                nc.scalar.activation(exp_e[:], e_ps[:], AF.Exp, accum_out=s1[:])
                eh = sc.tile([GB, 128], F32, tag=f"eh{g}")
                s2 = sc.tile([GB, 1], F32, tag=f"s2{g}")
                nc.vector.scalar_tensor_tensor(
                    eh[:], exp_e[:], 1.0, gd["hw"][:],
                    op0=OP.mult, op1=OP.mult, accum_out=s2[:])
                r1 = sc.tile([GB, 1], F32, tag=f"r1{g}")
                nc.vector.reciprocal(r1[:], s1[:])
                # x = s2*r1 + yp[:, t]  -> column 0 of xblk
                nc.vector.tensor_scalar(xblk[0:GB, 0:1], s2[:], r1[:],
                                        gd["yp"][:, t:t + 1],
                                        op0=OP.mult, op1=OP.add)
                nc.vector.transpose(hsx[64:96, 0:32], xblk[:])

                z2 = ps_z.tile([128, 2 * GB], F32, tag=f"z{g}")
                nc.tensor.matmul(z2[:, 0:GB], wrak[:], hsx[0:65, 0:GB],
                                 start=True, stop=True)
                nc.tensor.matmul(z2[:, GB:2 * GB], wrbk[:], hsx[0:65, 0:GB],
                                 start=True, stop=True)

                thg = sc.tile([128, 2 * GB], F32, tag=f"tg{g}")
                nc.scalar.activation(thg[:], z2[:], AF.Tanh)

                # cell: S_new = 0.5*(thf+1)*S + (thi+1)*g ; H_new = (tho+1)*tanh(S_new/2)
                asc = sc.tile([64, GB], F32, tag=f"a{g}")
                nc.vector.scalar_tensor_tensor(asc[:], thg[0:64, 0:GB], 1.0,
                                               thg[0:64, GB:2 * GB],
                                               op0=OP.add, op1=OP.mult)
                bsc = sc.tile([64, GB], F32, tag=f"b{g}")
                nc.vector.scalar_tensor_tensor(bsc[:], thg[64:128, 0:GB], 1.0,
                                               hs[64:128, :],
                                               op0=OP.add, op1=OP.mult)
                nc.vector.scalar_tensor_tensor(hs[64:128, :], bsc[:], 0.5, asc[:],
                                               op0=OP.mult, op1=OP.add)
                tcn = sc.tile([128, GB], F32, tag=f"tc{g}")
                nc.scalar.activation(tcn[64:128, :], hs[64:128, :], AF.Tanh, scale=0.5)
                nc.vector.scalar_tensor_tensor(hs[0:64, :], thg[64:128, GB:2 * GB],
                                               1.0, tcn[64:128, :],
                                               op0=OP.add, op1=OP.mult)

        # =================== final attention ===================
        for g in range(G):
            gd = grp[g]
            th = q_and_tanh(g)
            e_ps = ve_contract(g, th)
            exp_e = sc.tile([GB, 128], F32, tag=f"ex{g}")
            s1 = sc.tile([GB, 1], F32, tag=f"s1{g}")
            nc.scalar.activation(exp_e[:], e_ps[:], AF.Exp, accum_out=s1[:])
            r1 = sc.tile([GB, 1], F32, tag=f"r1{g}")
            nc.vector.reciprocal(r1[:], s1[:])

            eT_ps = ps_z.tile([128, GB], F32, tag=f"z{g}")
            for kl in range(GCH):
                nc.tensor.matmul(eT_ps[:, 2 * kl:2 * kl + 2],
                                 th[:, kl * 128:(kl + 1) * 128], vb2[:],
                                 start=True, stop=True)
            expT = sc.tile([128, GB], F32, tag=f"eT{g}")
            nc.scalar.activation(expT[:], eT_ps[:], AF.Exp)

            zp = stat.tile([128, GB * GB], F32, tag=f"zp{g}")
            nc.gpsimd.memset(zp[:], 0.0)
            for j in range(GB):
                nc.vector.tensor_copy(zp[:, j * GB + j:j * GB + j + 1],
                                      expT[:, j:j + 1])
            ctx_ps = ps_e.tile([GB, 64], F32, tag=f"e{g}")
            for j in range(GB):
                bglob = g * GB + j
                nc.tensor.matmul(ctx_ps[:], zp[:, j * GB:(j + 1) * GB],
                                 hen[:, bglob * 64:(bglob + 1) * 64],
                                 start=(j == 0), stop=(j == GB - 1))
            ctx_sb = sc.tile([GB, 64], F32, tag=f"cs{g}")
            nc.vector.tensor_scalar_mul(ctx_sb[:], ctx_ps[:], r1[:])
            nc.sync.dma_start(ctxout_d[g * GB:(g + 1) * GB, :], ctx_sb[:])

            hf_sb = sc.tile([64, GB], F32, tag=f"hf{g}")
            nc.scalar.mul(hf_sb[:], gd["hs"][0:64, :], 0.5)
            nc.sync.dma_start(
                hout_d[g * GB:(g + 1) * GB, :].rearrange("b u -> u b"), hf_sb[:])

    nc.compile()
    return nc


def get_program():
    if "nc" not in _prog_cache:
        _prog_cache["nc"] = build_program()
    return _prog_cache["nc"]


# ---------------------------------------------------------------- entry
def kernel(**inputs):
    _, _, _, _, run_bass_kernel_spmd = _imports()
    nc = get_program()
    in_maps = [host_prep(inputs, core) for core in range(N_CORES)]
    res = run_bass_kernel_spmd(nc, in_maps, list(range(N_CORES)))
    h_f = np.empty((B, U), np.float32)
    ctx = np.empty((B, 1, M), np.float32)
    for core in range(N_CORES):
        h_f[core * BS:(core + 1) * BS] = res.results[core]["h_out"]
        ctx[core * BS:(core + 1) * BS, 0] = res.results[core]["ctx_out"]
    return h_f, ctx


# revision 21
# speedup vs baseline: 1.2353x; 1.2353x over previous
"""Trainium2 Bass kernel for the attention-decoder recurrence.

Problem: B=128 batch, T=128 steps, U=M=64. A 127-step sequential scan of
(Bahdanau-attention -> 1-dim projection -> LSTM cell), then a final
attention readout. Returns (h_f (128,64), ctx (128,1,64)).

Sharding: data-parallel over batch, 16 per NeuronCore x 8 cores, all
parameters replicated, no cross-device traffic inside the recurrence.

Per-core structure (see kernel-side comments):
 - The 16-sample slice is further split into two independent groups of 8
   whose serial dependency chains interleave on the engines (software
   pipelining across the step's ~45 small ops).
 - Recurrent state per group: hs (128,8) SBUF, rows 0:64 H=2h
   (transposed), rows 64:128 S=2c. The factor-2 state convention turns
   every sigmoid into 0.5*tanh(z/2)+0.5 with the 0.5s folded into
   host-prescaled weights, so ScalarE only ever needs the exp/tanh table
   set (no ~2.7us table switches).
 - r2 = h_en@Ue is host-precomputed and staged into PSUM in a
   (p=(b%2)*64+u, k*128+t) layout; each per-step tanh ACT op reads one
   128x128 chunk with a per-partition bias column q (fused add+tanh).
 - e = ve . tanh(...) via PE matmuls with a zero-padded bf16 stationary
   operand; softmax needs no max-subtraction (|e| <= sum|ve| ~ 5).
 - x = y*Wp0 + bp + (sum exp*hw)/(sum exp) uses fused exp+accum (s1),
   fused scalar_tensor_tensor+accum (s2), and a two-scalar tensor_scalar.
 - LSTM gate preactivations via one K=65 stacked matmul per gate pair.
"""
import numpy as np

B, T, U, M = 128, 128, 64, 64
N_CORES = 8
BS = B // N_CORES   # 16 batch per core
G = 2               # pipeline groups per core
GB = BS // G        # 8 batch per group
KCH = BS // 2       # 8 tanh chunks total per core
GCH = KCH // G      # 4 chunks per group

_prog_cache = {}


def _imports():
    import concourse.bass as bass
    import concourse.tile as tile
    from concourse import bacc, mybir
    from concourse.bass_utils import run_bass_kernel_spmd
    return bass, tile, bacc, mybir, run_bass_kernel_spmd


# ---------------------------------------------------------------- host prep
def host_prep(inputs, core):
    """Host-side numpy preprocessing for one core's input map."""
    import ml_dtypes
    sl = slice(core * BS, (core + 1) * BS)
    h_en = np.asarray(inputs["h_en_all"][sl], np.float32)   # (16, 128, 64)
    y = np.asarray(inputs["y"][sl, :, 0], np.float32)       # (16, 128)
    s0 = np.asarray(inputs["s"][sl], np.float32)            # (16, 64)
    h0 = np.asarray(inputs["h"][sl], np.float32)            # (16, 64)
    We = np.asarray(inputs["We"], np.float32)               # (128, 64)
    Ue = np.asarray(inputs["Ue"], np.float32)               # (64, 64)
    ve = np.asarray(inputs["ve"], np.float32)[:, 0]         # (64,)
    Wp = np.asarray(inputs["Wp"], np.float32)[:, 0]         # (65,)
    bp = float(np.asarray(inputs["bp"], np.float32)[0])
    Wk = np.asarray(inputs["Wk"], np.float32)               # (1, 256)
    Wr = np.asarray(inputs["Wr"], np.float32)               # (64, 256)
    b = np.asarray(inputs["b"], np.float32)                 # (256,)
    assert np.abs(b).max() == 0.0, "nonzero LSTM bias not supported"

    d = {}
    r2 = np.einsum("btm,mu->btu", h_en, Ue).astype(np.float32)
    r2_l = np.empty((128, 1024), np.float32)
    for k in range(KCH):
        for bb in range(2):
            r2_l[bb * 64:(bb + 1) * 64, k * 128:(k + 1) * 128] = r2[2 * k + bb].T
    d["r2_l"] = r2_l
    d["hen_sb"] = np.ascontiguousarray(h_en.transpose(1, 0, 2).reshape(T, BS * M))
    d["we2"] = np.ascontiguousarray(We * 0.5)
    # zero-padded ve blocks, per group: (128, GCH*GB) and the (128,2) final
    ve_blkz = np.zeros((128, KCH * GB), np.float32)
    ve_blk2 = np.zeros((128, 2), np.float32)
    for p in range(128):
        bb, u = p // 64, p % 64
        for k in range(KCH):          # global chunk k covers b = 2k, 2k+1
            g, kl = divmod(k, GCH)
            j = 2 * k + bb - GB * g   # local batch index within group
            ve_blkz[p, (g * GCH + kl) * GB + j] = ve[u]
        ve_blk2[p, bb] = ve[u]
    d["ve_blkz"] = ve_blkz.astype(np.float16)
    d["ve_blk2"] = ve_blk2.astype(np.float16)
    wra = Wr[:, 0:128] * 0.25
    wrb = np.concatenate([Wr[:, 128:192] * 0.5, Wr[:, 192:256] * 0.25], axis=1)
    wka = Wk[:, 0:128] * 0.5
    wkb = np.concatenate([Wk[:, 128:192], Wk[:, 192:256] * 0.5], axis=1)
    d["wrak"] = np.concatenate([wra, wka], axis=0).astype(np.float16)  # (65,128)
    d["wrbk"] = np.concatenate([wrb, wkb], axis=0).astype(np.float16)  # (65,128)
    d["hw_t"] = np.einsum("btm,m->bt", h_en, Wp[1:]).astype(np.float32)
    d["yp"] = (y * Wp[0] + bp).astype(np.float32)
    d["hs0"] = np.concatenate([2.0 * h0.T, 2.0 * s0.T], axis=0).astype(np.float32)
    return d


# ---------------------------------------------------------------- program
def build_program(n_steps=T - 1):
    bass, tile, bacc, mybir, _ = _imports()
    from contextlib import ExitStack
    F32 = mybir.dt.float32
    F16 = mybir.dt.float16
    AF = mybir.ActivationFunctionType
    OP = mybir.AluOpType

    nc = bacc.Bacc("TRN2", target_bir_lowering=False, debug=False,
                   num_devices=N_CORES)

    def din(name, shape, dt=F32):
        return nc.dram_tensor(name, list(shape), dt, kind="ExternalInput").ap()

    r2_d = din("r2_l", (128, 1024))
    hen_d = din("hen_sb", (128, 1024))
    we2_d = din("we2", (128, 64))
    vbz_d = din("ve_blkz", (128, KCH * GB), F16)
    vb2_d = din("ve_blk2", (128, 2), F16)
    wrak_d = din("wrak", (65, 128), F16)
    wrbk_d = din("wrbk", (65, 128), F16)
    hw_d = din("hw_t", (16, 128))
    yp_d = din("yp", (16, 128))
    hs0_d = din("hs0", (128, 16))
    hout_d = nc.dram_tensor("h_out", [BS, U], F32, kind="ExternalOutput").ap()
    ctxout_d = nc.dram_tensor("ctx_out", [BS, M], F32, kind="ExternalOutput").ap()

    with tile.TileContext(nc) as tc, ExitStack() as ctx:
        stat = ctx.enter_context(tc.tile_pool(name="stat", bufs=1))
        sc = ctx.enter_context(tc.tile_pool(name="sc", bufs=2))
        ps_q = ctx.enter_context(tc.tile_pool(name="ps_q", bufs=1, space="PSUM"))
        ps_e = ctx.enter_context(tc.tile_pool(name="ps_e", bufs=2, space="PSUM"))
        ps_z = ctx.enter_context(tc.tile_pool(name="ps_z", bufs=1, space="PSUM"))

        # ---- static loads
        r2 = stat.tile([128, 1024], F32)
        nc.sync.dma_start(r2[:], r2_d[:])
        hen = stat.tile([128, 1024], F32)
        nc.sync.dma_start(hen[:], hen_d[:])
        we2 = stat.tile([128, 64], F32)
        nc.sync.dma_start(we2[:], we2_d[:])
        vbz = stat.tile([128, KCH * GB], F16)
        nc.sync.dma_start(vbz[:], vbz_d[:])
        vb2 = stat.tile([128, 2], F16)
        nc.sync.dma_start(vb2[:], vb2_d[:])
        wrak = stat.tile([65, 128], F16)
        nc.sync.dma_start(wrak[:], wrak_d[:])
        wrbk = stat.tile([65, 128], F16)
        nc.sync.dma_start(wrbk[:], wrbk_d[:])

        # HAM warm-up: a dense burst of matmuls so the PE clock-gate opens
        # (K=8/8); steady-state MM duty afterwards is enough to retain it.
        for w in range(14):
            e_warm = ps_e.tile([GB, 128], F32, tag=f"e{w % G}", name=f"ew{w}")
            nc.tensor.matmul(e_warm[:],
                             r2[:, (w % 8) * 128:(w % 8) * 128 + GB],
                             r2[:, (w % 8) * 128:((w % 8) + 1) * 128],
                             start=True, stop=True)

        grp = []
        for g in range(G):
            gd = {}
            gd["hw"] = stat.tile([GB, 128], F32, tag=f"hw{g}", name=f"hw{g}")
            nc.sync.dma_start(gd["hw"][:], hw_d[g * GB:(g + 1) * GB, :])
            gd["yp"] = stat.tile([GB, 128], F32, tag=f"yp{g}", name=f"yp{g}")
            nc.sync.dma_start(gd["yp"][:], yp_d[g * GB:(g + 1) * GB, :])
            gd["hs"] = stat.tile([128, GB], F32, tag=f"hs{g}", name=f"hs{g}")
            nc.sync.dma_start(gd["hs"][:], hs0_d[:, g * GB:(g + 1) * GB])
            gd["xblk"] = stat.tile([32, 32], F16, tag=f"xblk{g}", name=f"xblk{g}")
            nc.gpsimd.memset(gd["xblk"][:], 0.0)
            gd["hsx"] = stat.tile([96, 32], F16, tag=f"hsx{g}", name=f"hsx{g}")
            grp.append(gd)

        def q_and_tanh(g):
            """q even/odd MMs, psum->sbuf copy, GCH tanh chunks -> th (bf16)."""
            gd = grp[g]
            hs = gd["hs"]
            q2 = ps_q.tile([128, GB // 2], F32, tag=f"q{g}")
            nc.tensor.matmul(q2[0:64, :], we2[:], hs[:, 0::2], start=True, stop=True)
            nc.tensor.matmul(q2[64:128, :], we2[:], hs[:, 1::2], start=True, stop=True)
            q_sb = sc.tile([128, GB // 2], F32, tag=f"qs{g}")
            nc.vector.tensor_copy(q_sb[:], q2[:])
            th = sc.tile([128, GCH * 128], F16, tag=f"th{g}")
            for kl in range(GCH):
                kg = g * GCH + kl
                nc.scalar.activation(th[:, kl * 128:(kl + 1) * 128],
                                     r2[:, kg * 128:(kg + 1) * 128],
                                     AF.Tanh, bias=q_sb[:, kl:kl + 1], scale=1.0)
            return th

        def ve_contract(g, th):
            e_ps = ps_e.tile([GB, 128], F32, tag=f"e{g}")
            for kl in range(GCH):
                nc.tensor.matmul(
                    e_ps[:], vbz[:, (g * GCH + kl) * GB:(g * GCH + kl + 1) * GB],
                    th[:, kl * 128:(kl + 1) * 128],
                    start=(kl == 0), stop=(kl == GCH - 1))
            return e_ps

        # =================== the scan ===================
        for t in range(n_steps):
            for g in range(G):
                gd = grp[g]
                hs, xblk, hsx = gd["hs"], gd["xblk"], gd["hsx"]
                nc.vector.tensor_copy(hsx[0:64, 0:GB], hs[0:64, :])
                th = q_and_tanh(g)
                e_ps = ve_contract(g, th)

                exp_e = sc.tile([GB, 128], F32, tag=f"ex{g}")
                s1 = sc.tile([GB, 1], F32, tag=f"s1{g}")
                nc.scalar.activation(exp_e[:], e_ps[:], AF.Exp, accum_out=s1[:])
                eh = sc.tile([GB, 128], F32, tag=f"eh{g}")
                s2 = sc.tile([GB, 1], F32, tag=f"s2{g}")
                nc.vector.scalar_tensor_tensor(
                    eh[:], exp_e[:], 1.0, gd["hw"][:],
                    op0=OP.mult, op1=OP.mult, accum_out=s2[:])
                r1 = sc.tile([GB, 1], F32, tag=f"r1{g}")
                nc.vector.reciprocal(r1[:], s1[:])
                # x = s2*r1 + yp[:, t]  -> column 0 of xblk
                nc.vector.tensor_scalar(xblk[0:GB, 0:1], s2[:], r1[:],
                                        gd["yp"][:, t:t + 1],
                                        op0=OP.mult, op1=OP.add)
                nc.vector.transpose(hsx[64:96, 0:32], xblk[:])

                z2 = ps_z.tile([128, 2 * GB], F32, tag=f"z{g}")
                nc.tensor.matmul(z2[:, 0:GB], wrak[:], hsx[0:65, 0:GB],
                                 start=True, stop=True)
                nc.tensor.matmul(z2[:, GB:2 * GB], wrbk[:], hsx[0:65, 0:GB],
                                 start=True, stop=True)

                thg = sc.tile([128, 2 * GB], F32, tag=f"tg{g}")
                nc.scalar.activation(thg[:], z2[:], AF.Tanh)

                # cell: S_new = 0.5*(thf+1)*S + (thi+1)*g ; H_new = (tho+1)*tanh(S_new/2)
                asc = sc.tile([64, GB], F32, tag=f"a{g}")
                nc.vector.scalar_tensor_tensor(asc[:], thg[0:64, 0:GB], 1.0,
                                               thg[0:64, GB:2 * GB],
                                               op0=OP.add, op1=OP.mult)
                bsc = sc.tile([64, GB], F32, tag=f"b{g}")
                nc.vector.scalar_tensor_tensor(bsc[:], thg[64:128, 0:GB], 1.0,
                                               hs[64:128, :],
                                               op0=OP.add, op1=OP.mult)
                nc.vector.scalar_tensor_tensor(hs[64:128, :], bsc[:], 0.5, asc[:],
                                               op0=OP.mult, op1=OP.add)
                tcn = sc.tile([128, GB], F32, tag=f"tc{g}")
                nc.scalar.activation(tcn[64:128, :], hs[64:128, :], AF.Tanh, scale=0.5)
                nc.vector.scalar_tensor_tensor(hs[0:64, :], thg[64:128, GB:2 * GB],
                                               1.0, tcn[64:128, :],
                                               op0=OP.add, op1=OP.mult)

        # =================== final attention ===================
        for g in range(G):
            gd = grp[g]
            th = q_and_tanh(g)
            e_ps = ve_contract(g, th)
            exp_e = sc.tile([GB, 128], F32, tag=f"ex{g}")
            s1 = sc.tile([GB, 1], F32, tag=f"s1{g}")
            nc.scalar.activation(exp_e[:], e_ps[:], AF.Exp, accum_out=s1[:])
            r1 = sc.tile([GB, 1], F32, tag=f"r1{g}")
            nc.vector.reciprocal(r1[:], s1[:])

            eT_ps = ps_z.tile([128, GB], F32, tag=f"z{g}")
            for kl in range(GCH):
                nc.tensor.matmul(eT_ps[:, 2 * kl:2 * kl + 2],
                                 th[:, kl * 128:(kl + 1) * 128], vb2[:],
                                 start=True, stop=True)
            expT = sc.tile([128, GB], F32, tag=f"eT{g}")
            nc.scalar.activation(expT[:], eT_ps[:], AF.Exp)

            zp = stat.tile([128, GB * GB], F32, tag=f"zp{g}")
            nc.gpsimd.memset(zp[:], 0.0)
            for j in range(GB):
                nc.vector.tensor_copy(zp[:, j * GB + j:j * GB + j + 1],
                                      expT[:, j:j + 1])
            ctx_ps = ps_e.tile([GB, 64], F32, tag=f"e{g}")
            for j in range(GB):
                bglob = g * GB + j
                nc.tensor.matmul(ctx_ps[:], zp[:, j * GB:(j + 1) * GB],
                                 hen[:, bglob * 64:(bglob + 1) * 64],
                                 start=(j == 0), stop=(j == GB - 1))
            ctx_sb = sc.tile([GB, 64], F32, tag=f"cs{g}")
            nc.vector.tensor_scalar_mul(ctx_sb[:], ctx_ps[:], r1[:])
            nc.sync.dma_start(ctxout_d[g * GB:(g + 1) * GB, :], ctx_sb[:])

            hf_sb = sc.tile([64, GB], F32, tag=f"hf{g}")
            nc.scalar.mul(hf_sb[:], gd["hs"][0:64, :], 0.5)
            nc.sync.dma_start(
                hout_d[g * GB:(g + 1) * GB, :].rearrange("b u -> u b"), hf_sb[:])

    nc.compile()
    return nc


def get_program():
    if "nc" not in _prog_cache:
        _prog_cache["nc"] = build_program()
    return _prog_cache["nc"]


# ---------------------------------------------------------------- entry
def kernel(**inputs):
    _, _, _, _, run_bass_kernel_spmd = _imports()
    nc = get_program()
    in_maps = [host_prep(inputs, core) for core in range(N_CORES)]
    res = run_bass_kernel_spmd(nc, in_maps, list(range(N_CORES)))
    h_f = np.empty((B, U), np.float32)
    ctx = np.empty((B, 1, M), np.float32)
    for core in range(N_CORES):
        h_f[core * BS:(core + 1) * BS] = res.results[core]["h_out"]
        ctx[core * BS:(core + 1) * BS, 0] = res.results[core]["ctx_out"]
    return h_f, ctx


# revision 26
# speedup vs baseline: 1.2382x; 1.0023x over previous
"""Trainium2 Bass kernel for the attention-decoder recurrence.

Problem: B=128 batch, T=128 steps, U=M=64. A 127-step sequential scan of
(Bahdanau-attention -> 1-dim projection -> LSTM cell), then a final
attention readout. Returns (h_f (128,64), ctx (128,1,64)).

Sharding: data-parallel over batch, 16 per NeuronCore x 8 cores, all
parameters replicated, no cross-device traffic inside the recurrence.

Per-core structure (see kernel-side comments):
 - The 16-sample slice is further split into two independent groups of 8
   whose serial dependency chains interleave on the engines (software
   pipelining across the step's ~45 small ops).
 - Recurrent state per group: hs (128,8) SBUF, rows 0:64 H=2h
   (transposed), rows 64:128 S=2c. The factor-2 state convention turns
   every sigmoid into 0.5*tanh(z/2)+0.5 with the 0.5s folded into
   host-prescaled weights, so ScalarE only ever needs the exp/tanh table
   set (no ~2.7us table switches).
 - r2 = h_en@Ue is host-precomputed and staged into SBUF in a
   (p=(b%2)*64+u, k*128+t) layout; each per-step tanh ACT op reads one
   128x128 chunk with a per-partition bias column q (fused add+tanh).
 - e = ve . tanh(...) via PE matmuls with a zero-padded fp16 stationary
   operand (fp16 streams at 1 cycle/row like bf16 but carries a 10-bit
   mantissa; fp32 matmuls lower to 2 half-rate passes and LDWEIGHTS is
   not hidden in this pipeline, so narrow dtypes double PE throughput).
   Softmax needs no max-subtraction (|e| <= sum|ve| ~ 5).
 - x = y*Wp0 + bp + (sum exp*hw)/(sum exp) uses fused exp+accum (s1),
   fused scalar_tensor_tensor+accum (s2), and a two-scalar tensor_scalar.
 - LSTM gate preactivations via one K=65 stacked fp16 matmul per gate
   pair; the recurrent state itself stays fp32 (end-to-end rel err vs
   the fp32 reference is ~4e-4, dominated by the fp16 tanh rounding).
 - A dense warm-up burst of matmuls at kernel start opens the PE HAM
   clock gate (cold PE runs at 1.2 GHz vs 2.4 GHz warm).
"""
import numpy as np

B, T, U, M = 128, 128, 64, 64
N_CORES = 8
BS = B // N_CORES   # 16 batch per core
G = 2               # pipeline groups per core
GB = BS // G        # 8 batch per group
KCH = BS // 2       # 8 tanh chunks total per core
GCH = KCH // G      # 4 chunks per group

_prog_cache = {}


def _imports():
    import concourse.bass as bass
    import concourse.tile as tile
    from concourse import bacc, mybir
    from concourse.bass_utils import run_bass_kernel_spmd
    return bass, tile, bacc, mybir, run_bass_kernel_spmd


# ---------------------------------------------------------------- host prep
def host_prep(inputs, core):
    """Host-side numpy preprocessing for one core's input map."""
    sl = slice(core * BS, (core + 1) * BS)
    h_en = np.asarray(inputs["h_en_all"][sl], np.float32)   # (16, 128, 64)
    y = np.asarray(inputs["y"][sl, :, 0], np.float32)       # (16, 128)
    s0 = np.asarray(inputs["s"][sl], np.float32)            # (16, 64)
    h0 = np.asarray(inputs["h"][sl], np.float32)            # (16, 64)
    We = np.asarray(inputs["We"], np.float32)               # (128, 64)
    Ue = np.asarray(inputs["Ue"], np.float32)               # (64, 64)
    ve = np.asarray(inputs["ve"], np.float32)[:, 0]         # (64,)
    Wp = np.asarray(inputs["Wp"], np.float32)[:, 0]         # (65,)
    bp = float(np.asarray(inputs["bp"], np.float32)[0])
    Wk = np.asarray(inputs["Wk"], np.float32)               # (1, 256)
    Wr = np.asarray(inputs["Wr"], np.float32)               # (64, 256)
    b = np.asarray(inputs["b"], np.float32)                 # (256,)
    assert np.abs(b).max() == 0.0, "nonzero LSTM bias not supported"

    d = {}
    r2 = np.einsum("btm,mu->btu", h_en, Ue).astype(np.float32)
    r2_l = np.empty((128, 1024), np.float32)
    for k in range(KCH):
        for bb in range(2):
            r2_l[bb * 64:(bb + 1) * 64, k * 128:(k + 1) * 128] = r2[2 * k + bb].T
    d["r2_l"] = r2_l
    d["hen_sb"] = np.ascontiguousarray(h_en.transpose(1, 0, 2).reshape(T, BS * M))
    d["we2"] = np.ascontiguousarray(We * 0.5)
    # zero-padded ve blocks, per group: (128, GCH*GB) and the (128,2) final
    ve_blkz = np.zeros((128, KCH * GB), np.float32)
    ve_blk2 = np.zeros((128, 2), np.float32)
    for p in range(128):
        bb, u = p // 64, p % 64
        for k in range(KCH):          # global chunk k covers b = 2k, 2k+1
            g, kl = divmod(k, GCH)
            j = 2 * k + bb - GB * g   # local batch index within group
            ve_blkz[p, (g * GCH + kl) * GB + j] = ve[u]
        ve_blk2[p, bb] = ve[u]
    d["ve_blkz"] = ve_blkz.astype(np.float16)
    d["ve_blk2"] = ve_blk2.astype(np.float16)
    wra = Wr[:, 0:128] * 0.25
    wrb = np.concatenate([Wr[:, 128:192] * 0.5, Wr[:, 192:256] * 0.25], axis=1)
    wka = Wk[:, 0:128] * 0.5
    wkb = np.concatenate([Wk[:, 128:192], Wk[:, 192:256] * 0.5], axis=1)
    d["wrak"] = np.concatenate([wra, wka], axis=0).astype(np.float16)  # (65,128)
    d["wrbk"] = np.concatenate([wrb, wkb], axis=0).astype(np.float16)  # (65,128)
    d["hw_t"] = np.einsum("btm,m->bt", h_en, Wp[1:]).astype(np.float32)
    d["yp"] = (y * Wp[0] + bp).astype(np.float32)
    d["hs0"] = np.concatenate([2.0 * h0.T, 2.0 * s0.T], axis=0).astype(np.float32)
    return d


# ---------------------------------------------------------------- program
def build_program(n_steps=T - 1):
    bass, tile, bacc, mybir, _ = _imports()
    from contextlib import ExitStack
    F32 = mybir.dt.float32
    F16 = mybir.dt.float16
    AF = mybir.ActivationFunctionType
    OP = mybir.AluOpType

    nc = bacc.Bacc("TRN2", target_bir_lowering=False, debug=False,
                   num_devices=N_CORES)

    def din(name, shape, dt=F32):
        return nc.dram_tensor(name, list(shape), dt, kind="ExternalInput").ap()

    r2_d = din("r2_l", (128, 1024))
    hen_d = din("hen_sb", (128, 1024))
    we2_d = din("we2", (128, 64))
    vbz_d = din("ve_blkz", (128, KCH * GB), F16)
    vb2_d = din("ve_blk2", (128, 2), F16)
    wrak_d = din("wrak", (65, 128), F16)
    wrbk_d = din("wrbk", (65, 128), F16)
    hw_d = din("hw_t", (16, 128))
    yp_d = din("yp", (16, 128))
    hs0_d = din("hs0", (128, 16))
    hout_d = nc.dram_tensor("h_out", [BS, U], F32, kind="ExternalOutput").ap()
    ctxout_d = nc.dram_tensor("ctx_out", [BS, M], F32, kind="ExternalOutput").ap()

    with tile.TileContext(nc) as tc, ExitStack() as ctx:
        stat = ctx.enter_context(tc.tile_pool(name="stat", bufs=1))
        sc = ctx.enter_context(tc.tile_pool(name="sc", bufs=2))
        ps_q = ctx.enter_context(tc.tile_pool(name="ps_q", bufs=1, space="PSUM"))
        ps_e = ctx.enter_context(tc.tile_pool(name="ps_e", bufs=2, space="PSUM"))
        ps_z = ctx.enter_context(tc.tile_pool(name="ps_z", bufs=1, space="PSUM"))

        # ---- static loads
        r2 = stat.tile([128, 1024], F32)
        nc.sync.dma_start(r2[:], r2_d[:])
        hen = stat.tile([128, 1024], F32)
        nc.sync.dma_start(hen[:], hen_d[:])
        we2 = stat.tile([128, 64], F32)
        nc.sync.dma_start(we2[:], we2_d[:])
        vbz = stat.tile([128, KCH * GB], F16)
        nc.sync.dma_start(vbz[:], vbz_d[:])
        vb2 = stat.tile([128, 2], F16)
        nc.sync.dma_start(vb2[:], vb2_d[:])
        wrak = stat.tile([65, 128], F16)
        nc.sync.dma_start(wrak[:], wrak_d[:])
        wrbk = stat.tile([65, 128], F16)
        nc.sync.dma_start(wrbk[:], wrbk_d[:])

        # HAM warm-up: a dense burst of matmuls so the PE clock-gate opens
        # (K=8/8); steady-state MM duty afterwards is enough to retain it.
        for w in range(14):
            e_warm = ps_e.tile([GB, 128], F32, tag=f"e{w % G}", name=f"ew{w}")
            nc.tensor.matmul(e_warm[:],
                             r2[:, (w % 8) * 128:(w % 8) * 128 + GB],
                             r2[:, (w % 8) * 128:((w % 8) + 1) * 128],
                             start=True, stop=True)

        grp = []
        for g in range(G):
            gd = {}
            gd["hw"] = stat.tile([GB, 128], F32, tag=f"hw{g}", name=f"hw{g}")
            nc.sync.dma_start(gd["hw"][:], hw_d[g * GB:(g + 1) * GB, :])
            gd["yp"] = stat.tile([GB, 128], F32, tag=f"yp{g}", name=f"yp{g}")
            nc.sync.dma_start(gd["yp"][:], yp_d[g * GB:(g + 1) * GB, :])
            gd["hs"] = stat.tile([128, GB], F32, tag=f"hs{g}", name=f"hs{g}")
            nc.sync.dma_start(gd["hs"][:], hs0_d[:, g * GB:(g + 1) * GB])
            gd["xblk"] = stat.tile([32, 32], F16, tag=f"xblk{g}", name=f"xblk{g}")
            nc.gpsimd.memset(gd["xblk"][:], 0.0)
            gd["hsx"] = stat.tile([96, 32], F16, tag=f"hsx{g}", name=f"hsx{g}")
            grp.append(gd)

        def q_and_tanh(g):
            """q even/odd MMs, psum->sbuf copy, GCH tanh chunks -> th (bf16)."""
            gd = grp[g]
            hs = gd["hs"]
            q2 = ps_q.tile([128, GB // 2], F32, tag=f"q{g}")
            nc.tensor.matmul(q2[0:64, :], we2[:], hs[:, 0::2], start=True, stop=True)
            nc.tensor.matmul(q2[64:128, :], we2[:], hs[:, 1::2], start=True, stop=True)
            q_sb = sc.tile([128, GB // 2], F32, tag=f"qs{g}")
            nc.vector.tensor_copy(q_sb[:], q2[:])
            th = sc.tile([128, GCH * 128], F16, tag=f"th{g}")
            for kl in range(GCH):
                kg = g * GCH + kl
                nc.scalar.activation(th[:, kl * 128:(kl + 1) * 128],
                                     r2[:, kg * 128:(kg + 1) * 128],
                                     AF.Tanh, bias=q_sb[:, kl:kl + 1], scale=1.0)
            return th

        def ve_contract(g, th):
            e_ps = ps_e.tile([GB, 128], F32, tag=f"e{g}")
            for kl in range(GCH):
                nc.tensor.matmul(
                    e_ps[:], vbz[:, (g * GCH + kl) * GB:(g * GCH + kl + 1) * GB],
                    th[:, kl * 128:(kl + 1) * 128],
                    start=(kl == 0), stop=(kl == GCH - 1))
            return e_ps

        # =================== the scan ===================
        for t in range(n_steps):
            for g in range(G):
                gd = grp[g]
                hs, xblk, hsx = gd["hs"], gd["xblk"], gd["hsx"]
                nc.vector.tensor_copy(hsx[0:64, 0:GB], hs[0:64, :])
                th = q_and_tanh(g)
                e_ps = ve_contract(g, th)

                exp_e = sc.tile([GB, 128], F32, tag=f"ex{g}")
                s1 = sc.tile([GB, 1], F32, tag=f"s1{g}")
                nc.scalar.activation(exp_e[:], e_ps[:], AF.Exp, accum_out=s1[:])
                eh = sc.tile([GB, 128], F32, tag=f"eh{g}")
                s2 = sc.tile([GB, 1], F32, tag=f"s2{g}")
                nc.vector.scalar_tensor_tensor(
                    eh[:], exp_e[:], 1.0, gd["hw"][:],
                    op0=OP.mult, op1=OP.mult, accum_out=s2[:])
                r1 = sc.tile([GB, 1], F32, tag=f"r1{g}")
                nc.vector.reciprocal(r1[:], s1[:])
                # x = s2*r1 + yp[:, t]  -> column 0 of xblk
                nc.vector.tensor_scalar(xblk[0:GB, 0:1], s2[:], r1[:],
                                        gd["yp"][:, t:t + 1],
                                        op0=OP.mult, op1=OP.add)
                nc.vector.transpose(hsx[64:96, 0:32], xblk[:])

                z2 = ps_z.tile([128, 2 * GB], F32, tag=f"z{g}")
                nc.tensor.matmul(z2[:, 0:GB], wrak[:], hsx[0:65, 0:GB],
                                 start=True, stop=True)
                nc.tensor.matmul(z2[:, GB:2 * GB], wrbk[:], hsx[0:65, 0:GB],
                                 start=True, stop=True)

                thg = sc.tile([128, 2 * GB], F32, tag=f"tg{g}")
                nc.scalar.activation(thg[:], z2[:], AF.Tanh)

                # cell: S_new = 0.5*(thf+1)*S + (thi+1)*g ; H_new = (tho+1)*tanh(S_new/2)
                asc = sc.tile([64, GB], F32, tag=f"a{g}")
                nc.vector.scalar_tensor_tensor(asc[:], thg[0:64, 0:GB], 1.0,
                                               thg[0:64, GB:2 * GB],
                                               op0=OP.add, op1=OP.mult)
                bsc = sc.tile([64, GB], F32, tag=f"b{g}")
                nc.vector.scalar_tensor_tensor(bsc[:], thg[64:128, 0:GB], 1.0,
                                               hs[64:128, :],
                                               op0=OP.add, op1=OP.mult)
                nc.vector.scalar_tensor_tensor(hs[64:128, :], bsc[:], 0.5, asc[:],
                                               op0=OP.mult, op1=OP.add)
                tcn = sc.tile([128, GB], F32, tag=f"tc{g}")
                nc.scalar.activation(tcn[64:128, :], hs[64:128, :], AF.Tanh, scale=0.5)
                nc.vector.scalar_tensor_tensor(hs[0:64, :], thg[64:128, GB:2 * GB],
                                               1.0, tcn[64:128, :],
                                               op0=OP.add, op1=OP.mult)

        # =================== final attention ===================
        for g in range(G):
            gd = grp[g]
            th = q_and_tanh(g)
            e_ps = ve_contract(g, th)
            exp_e = sc.tile([GB, 128], F32, tag=f"ex{g}")
            s1 = sc.tile([GB, 1], F32, tag=f"s1{g}")
            nc.scalar.activation(exp_e[:], e_ps[:], AF.Exp, accum_out=s1[:])
            r1 = sc.tile([GB, 1], F32, tag=f"r1{g}")
            nc.vector.reciprocal(r1[:], s1[:])

            eT_ps = ps_z.tile([128, GB], F32, tag=f"z{g}")
            for kl in range(GCH):
                nc.tensor.matmul(eT_ps[:, 2 * kl:2 * kl + 2],
                                 th[:, kl * 128:(kl + 1) * 128], vb2[:],
                                 start=True, stop=True)
            expT = sc.tile([128, GB], F32, tag=f"eT{g}")
            nc.scalar.activation(expT[:], eT_ps[:], AF.Exp)

            zp = stat.tile([128, GB * GB], F32, tag=f"zp{g}")
            nc.gpsimd.memset(zp[:], 0.0)
            for j in range(GB):
                nc.vector.tensor_copy(zp[:, j * GB + j:j * GB + j + 1],
                                      expT[:, j:j + 1])
            ctx_ps = ps_e.tile([GB, 64], F32, tag=f"e{g}")
            for j in range(GB):
                bglob = g * GB + j
                nc.tensor.matmul(ctx_ps[:], zp[:, j * GB:(j + 1) * GB],
                                 hen[:, bglob * 64:(bglob + 1) * 64],
                                 start=(j == 0), stop=(j == GB - 1))
            ctx_sb = sc.tile([GB, 64], F32, tag=f"cs{g}")
            nc.vector.tensor_scalar_mul(ctx_sb[:], ctx_ps[:], r1[:])
            nc.sync.dma_start(ctxout_d[g * GB:(g + 1) * GB, :], ctx_sb[:])

            hf_sb = sc.tile([64, GB], F32, tag=f"hf{g}")
            nc.scalar.mul(hf_sb[:], gd["hs"][0:64, :], 0.5)
            nc.sync.dma_start(
                hout_d[g * GB:(g + 1) * GB, :].rearrange("b u -> u b"), hf_sb[:])

    nc.compile()
    return nc


def get_program():
    if "nc" not in _prog_cache:
        _prog_cache["nc"] = build_program()
    return _prog_cache["nc"]


# ---------------------------------------------------------------- entry
def kernel(**inputs):
    _, _, _, _, run_bass_kernel_spmd = _imports()
    nc = get_program()
    in_maps = [host_prep(inputs, core) for core in range(N_CORES)]
    res = run_bass_kernel_spmd(nc, in_maps, list(range(N_CORES)))
    h_f = np.empty((B, U), np.float32)
    ctx = np.empty((B, 1, M), np.float32)
    for core in range(N_CORES):
        h_f[core * BS:(core + 1) * BS] = res.results[core]["h_out"]
        ctx[core * BS:(core + 1) * BS, 0] = res.results[core]["ctx_out"]
    return h_f, ctx
